# revision 7
# baseline (speedup 1.0000x reference)
"""Bahdanau attention Trainium2 kernel.

Problem (fp32): B=32, T=2048, H=1024
  q_proj = query @ Wq.T + bq                  [B, H]
  v_proj = values @ Wv.T + bv                 [B, T, H]
  score  = tanh(q_proj[:,None,:] + v_proj) @ Vw.T + Vb   [B, T, 1]
  attn   = softmax(score, axis=1)
  ctx    = sum_t attn * values                [B, H]
  returns (ctx, attn)

Sharding: data-parallel over batch, 4 batches per core on 8 cores.
Weights replicated. No collectives.

Per-core dataflow (all matmuls in fp32r = full PE rate):
  - values streamed in t-chunks of 512 (natural layout [t,h], SWDGE
    DMA-cast fp32->fp32r); PE-transposes 128x128 blocks into [h,t]
    layout for the projection matmul; the natural tiles stay resident
    in SBUF and are reused by the context matmul (values read from HBM
    exactly once).
  - v_projT accumulated per 128-o-tile in PSUM (WvT stationary);
    ScalarE applies tanh + (q_projT + bq + bv) bias directly
    PSUM->SBUF; score matmul (M=1) accumulates over o-tiles.
  - exact softmax per batch: reduce_max over [1,2048] scores, single
    ScalarE Exp with accum_out giving the denominator.
  - context matmul: attn column tiles (PE outer-product transpose)
    against resident natural value tiles, contraction over t in PSUM.
"""

import sys

sys.path.insert(0, "/opt/trn_rl_repo")

import numpy as np

import concourse.bass as bass
import concourse.mybir as mybir
import concourse.tile as tile
from concourse import bacc, bass_utils
from concourse.masks import make_identity

dt = mybir.dt
AF = mybir.ActivationFunctionType

B, T, H = 32, 2048, 1024
N_CORES = 8
BPC = B // N_CORES          # batches per core
P = 128
HT = H // P                 # 8 h-tiles (contraction)
OT = H // P                 # 8 o-tiles (projection output)
CHUNK = 512
NCHUNK = T // CHUNK         # 4 chunks per batch
SUB = CHUNK // P            # 4 t-subtiles per chunk
TS = T // P                 # 16 t-subtiles per batch
NAT_BUFS = 20               # resident naturals (16 per batch) + prefetch


def build(nc):
    q_d = nc.dram_tensor("query", [BPC, H], dt.float32, kind="ExternalInput").ap()
    v_d = nc.dram_tensor("values", [BPC, T, H], dt.float32, kind="ExternalInput").ap()
    wq_d = nc.dram_tensor("Wq", [H, H], dt.float32, kind="ExternalInput").ap()
    bq_d = nc.dram_tensor("bq", [H], dt.float32, kind="ExternalInput").ap()
    wv_d = nc.dram_tensor("Wv", [H, H], dt.float32, kind="ExternalInput").ap()
    bv_d = nc.dram_tensor("bv", [H], dt.float32, kind="ExternalInput").ap()
    vw_d = nc.dram_tensor("Vw", [1, H], dt.float32, kind="ExternalInput").ap()
    vb_d = nc.dram_tensor("Vb", [1], dt.float32, kind="ExternalInput").ap()
    ctx_d = nc.dram_tensor("ctx_out", [BPC, H], dt.float32, kind="ExternalOutput").ap()
    attn_d = nc.dram_tensor("attn_out", [BPC, T], dt.float32, kind="ExternalOutput").ap()

    with tile.TileContext(nc) as tc:
        with (
            tc.tile_pool(name="nat", bufs=NAT_BUFS) as nat_p,
            tc.tile_pool(name="valT", bufs=2) as valT_p,
            tc.tile_pool(name="wT", bufs=1) as wT_p,
            tc.tile_pool(name="tanh", bufs=3) as tanh_p,
            tc.tile_pool(name="row", bufs=1) as row_p,
            tc.tile_pool(name="smalls", bufs=1) as small_p,
            tc.tile_pool(name="ps_tp", bufs=2, space="PSUM") as ps_tp,
            tc.tile_pool(name="ps_vp", bufs=2, space="PSUM") as ps_vp,
            tc.tile_pool(name="ps_sc", bufs=1, space="PSUM") as ps_sc,
            tc.tile_pool(name="ps_at", bufs=1, space="PSUM") as ps_at,
            tc.tile_pool(name="ps_ctx", bufs=1, space="PSUM") as ps_ctx,
        ):
            # ---------- constants ----------
            ident_f = small_p.tile([P, P], dt.float32, tag="ident_f")
            make_identity(nc, ident_f[:])
            ident_r = small_p.tile([P, P], dt.float32r, tag="ident_r")
            nc.vector.tensor_copy(ident_r[:], ident_f[:])
            ones_t = small_p.tile([1, 1], dt.float32, tag="ones")
            nc.vector.memset(ones_t[:], 1.0)
            vb_t = small_p.tile([1, 1], dt.float32, tag="vb")
            nc.sync.dma_start(vb_t[:], vb_d[None, :])

            # biases in transposed o-layout [128, OT] (o = ot*128 + p)
            bq_t = small_p.tile([P, OT], dt.float32, tag="bq")
            nc.sync.dma_start(bq_t[:], bq_d.rearrange("(ot p) -> p ot", p=P))
            bv_t = small_p.tile([P, OT], dt.float32, tag="bv")
            nc.sync.dma_start(bv_t[:], bv_d.rearrange("(ot p) -> p ot", p=P))
            bqbv = small_p.tile([P, OT], dt.float32, tag="bqbv")
            nc.vector.tensor_add(bqbv[:], bq_t[:], bv_t[:])

            # VwT in [128, OT] layout, rounded to fp32r
            vwT_f = small_p.tile([P, OT], dt.float32, tag="vwT_f")
            nc.sync.dma_start(vwT_f[:], vw_d.rearrange("one (ot p) -> p (one ot)", p=P))
            vwT = small_p.tile([P, OT], dt.float32r, tag="vwT")
            nc.vector.tensor_copy(vwT[:], vwT_f[:])

            # queryT [128, HT, BPC] fp32r (h = hk*128 + p)
            qT = small_p.tile([P, HT, BPC], dt.float32r, tag="qT")
            for b in range(BPC):
                nc.gpsimd.dma_start(
                    qT[:, :, b], q_d[b].rearrange("(hk p) -> p hk", p=P)
                )

            # ---------- weight transposes ----------
            def load_wT(w_dram, tagname):
                tiles = []
                for hk in range(HT):
                    wt = wT_p.tile([P, H], dt.float32r, tag=f"{tagname}{hk}",
                                   name=f"{tagname}{hk}")
                    tiles.append(wt)
                for ot in range(OT):
                    wn = nat_p.tile([P, H], dt.float32r, tag="nat", name="wnat")
                    nc.gpsimd.dma_start(wn[:], w_dram[ot * P:(ot + 1) * P, :])
                    for hk in range(HT):
                        tp = ps_tp.tile([P, P], dt.float32r, tag="tp", name="wtp")
                        nc.tensor.transpose(tp[:], wn[:, hk * P:(hk + 1) * P], ident_r[:])
                        nc.vector.tensor_copy(tiles[hk][:, ot * P:(ot + 1) * P], tp[:])
                return tiles

            # ---------- q_proj + bias addend [128, OT, BPC] ----------
            # WqT blocks are transposed on the fly into a rotating buffer
            # (no resident WqT: saves 32KB/partition of SBUF).
            addend = small_p.tile([P, OT, BPC], dt.float32, tag="addend")
            wq_nats = []
            for ot in range(OT):
                wqn = nat_p.tile([P, H], dt.float32r, tag="nat", name="wqn")
                nc.gpsimd.dma_start(wqn[:], wq_d[ot * P:(ot + 1) * P, :])
                wq_nats.append(wqn)
            for ot in range(OT):
                qp = ps_vp.tile([P, BPC], dt.float32, tag="vp", name="qp")
                for hk in range(HT):
                    tp = ps_tp.tile([P, P], dt.float32r, tag="tp", name="wtp")
                    nc.tensor.transpose(
                        tp[:], wq_nats[ot][:, hk * P:(hk + 1) * P], ident_r[:]
                    )
                    wqt = valT_p.tile([P, P], dt.float32r, tag="wqt", name="wqt")
                    nc.vector.tensor_copy(wqt[:], tp[:])
                    nc.tensor.matmul(
                        qp[:], wqt[:], qT[:, hk, :],
                        start=(hk == 0), stop=(hk == HT - 1),
                    )
                nc.vector.tensor_scalar(
                    addend[:, ot, :], qp[:], bqbv[:, ot:ot + 1], None,
                    op0=mybir.AluOpType.add,
                )

            wvT = load_wT(wv_d, "wvT")

            # ---------- main loop ----------
            deferred = [None]

            def phase_a(b):
                """Returns (scores tile [1, T], list of natural tiles)."""
                scores = row_p.tile([1, T], dt.float32, tag="scores", name="scores")
                nats = []
                for c in range(NCHUNK):
                    cn = []
                    for s in range(SUB):
                        t0 = c * CHUNK + s * P
                        vn = nat_p.tile([P, H], dt.float32r, tag="nat", name="vnat")
                        nc.gpsimd.dma_start(vn[:], v_d[b, t0:t0 + P, :])
                        cn.append(vn)
                    nats.extend(cn)

                    # transpose chunk into [h, t] tiles
                    vT = []
                    for hk in range(HT):
                        vt = valT_p.tile([P, CHUNK], dt.float32r, tag=f"vT{hk}",
                                         name=f"vT{hk}")
                        tp = ps_tp.tile([P, CHUNK], dt.float32r, tag="tp", name="vtp")
                        for s in range(SUB):
                            nc.tensor.transpose(
                                tp[:, s * P:(s + 1) * P],
                                cn[s][:, hk * P:(hk + 1) * P],
                                ident_r[:],
                            )
                        nc.vector.tensor_copy(vt[:], tp[:])
                        vT.append(vt)

                    # projection + tanh + score
                    sc = ps_sc.tile([1, CHUNK], dt.float32, tag="sc", name="sc")
                    for ot in range(OT):
                        vp = ps_vp.tile([P, CHUNK], dt.float32, tag="vp", name="vp")
                        for hk in range(HT):
                            nc.tensor.matmul(
                                vp[:], wvT[hk][:, ot * P:(ot + 1) * P], vT[hk][:],
                                start=(hk == 0), stop=(hk == HT - 1),
                            )
                        th = tanh_p.tile([P, CHUNK], dt.float32r, tag="th", name="th")
                        nc.scalar.activation(
                            th[:], vp[:], AF.Tanh, bias=addend[:, ot, b:b + 1],
                        )
                        nc.tensor.matmul(
                            sc[:], vwT[:, ot:ot + 1], th[:],
                            start=(ot == 0), stop=(ot == OT - 1),
                        )
                    nc.vector.tensor_copy(scores[:, c * CHUNK:(c + 1) * CHUNK], sc[:])

                    if c == 0 and deferred[0] is not None:
                        deferred[0]()
                        deferred[0] = None
                return scores, nats

            def phase_b_scalar(b, scores):
                m_t = row_p.tile([1, 1], dt.float32, tag="m", name="m")
                nc.vector.reduce_max(m_t[:], scores[:], axis=mybir.AxisListType.X)
                bias_t = row_p.tile([1, 1], dt.float32, tag="bias", name="bias")
                nc.vector.tensor_scalar(
                    bias_t[:], vb_t[:], m_t[:], None, op0=mybir.AluOpType.subtract,
                )
                attn_u = row_p.tile([1, T], dt.float32, tag="attn_u", name="attn_u")
                den = row_p.tile([1, 1], dt.float32, tag="den", name="den")
                nc.scalar.activation(
                    attn_u[:], scores[:], AF.Exp, bias=bias_t[:], accum_out=den[:],
                )
                rden = row_p.tile([1, 1], dt.float32, tag="rden", name="rden")
                nc.vector.reciprocal(rden[:], den[:])
                return attn_u, rden

            def phase_b_pe(b, attn_u, rden, nats):
                at_ps = ps_at.tile([P, TS], dt.float32, tag="at", name="at")
                for ts in range(TS):
                    nc.tensor.matmul(
                        at_ps[:, ts:ts + 1], attn_u[:, ts * P:(ts + 1) * P], ones_t[:],
                        start=True, stop=True,
                    )
                atT = row_p.tile([P, TS], dt.float32r, tag="atT", name="atT")
                nc.vector.tensor_copy(atT[:], at_ps[:])

                cps = []
                for hh in range(2):
                    cp = ps_ctx.tile([1, 512], dt.float32, tag=f"ctx{hh}", name=f"ctx{hh}")
                    cps.append(cp)
                for ts in range(TS):
                    for hh in range(2):
                        nc.tensor.matmul(
                            cps[hh][:], atT[:, ts:ts + 1],
                            nats[ts][:, hh * 512:(hh + 1) * 512],
                            start=(ts == 0), stop=(ts == TS - 1),
                        )
                ctx_sb = row_p.tile([1, H], dt.float32, tag="ctx_sb", name="ctx_sb")
                for hh in range(2):
                    nc.vector.tensor_scalar(
                        ctx_sb[:, hh * 512:(hh + 1) * 512], cps[hh][:], rden[:], None,
                        op0=mybir.AluOpType.mult,
                    )
                nc.sync.dma_start(ctx_d[b:b + 1, :], ctx_sb[:])
                attn_n = row_p.tile([1, T], dt.float32, tag="attn_n", name="attn_n")
                nc.vector.tensor_scalar(
                    attn_n[:], attn_u[:], rden[:], None, op0=mybir.AluOpType.mult,
                )
                nc.sync.dma_start(attn_d[b:b + 1, :], attn_n[:])

            for b in range(BPC):
                scores, nats = phase_a(b)
                attn_u, rden = phase_b_scalar(b, scores)
                deferred[0] = (
                    lambda b=b, attn_u=attn_u, rden=rden, nats=nats:
                    phase_b_pe(b, attn_u, rden, nats)
                )
            deferred[0]()
            deferred[0] = None

    return nc


_cached = None


def _get_nc():
    global _cached
    if _cached is None:
        nc = bacc.Bacc("TRN2", target_bir_lowering=False, debug=False,
                       num_devices=N_CORES)
        build(nc)
        nc.compile()
        _cached = nc
    return _cached


def kernel(query, values, Wq, bq, Wv, bv, Vw, Vb):
    query = np.ascontiguousarray(query, dtype=np.float32)
    values = np.ascontiguousarray(values, dtype=np.float32)
    nc = _get_nc()
    in_maps = []
    for k in range(N_CORES):
        sl = slice(k * BPC, (k + 1) * BPC)
        in_maps.append({
            "query": query[sl],
            "values": values[sl],
            "Wq": np.ascontiguousarray(Wq, dtype=np.float32),
            "bq": np.ascontiguousarray(bq, dtype=np.float32),
            "Wv": np.ascontiguousarray(Wv, dtype=np.float32),
            "bv": np.ascontiguousarray(bv, dtype=np.float32),
            "Vw": np.ascontiguousarray(Vw, dtype=np.float32),
            "Vb": np.ascontiguousarray(Vb, dtype=np.float32),
        })
    res = bass_utils.run_bass_kernel_spmd(nc, in_maps, list(range(N_CORES)))
    ctx = np.concatenate([res.results[k]["ctx_out"] for k in range(N_CORES)], axis=0)
    attn = np.concatenate([res.results[k]["attn_out"] for k in range(N_CORES)], axis=0)
    return ctx, attn[:, :, None]


# revision 9
# speedup vs baseline: 18698.6368x; 18698.6368x over previous
"""Bahdanau attention Trainium2 kernel.

Problem (fp32): B=32, T=2048, H=1024
  q_proj = query @ Wq.T + bq                  [B, H]
  v_proj = values @ Wv.T + bv                 [B, T, H]
  score  = tanh(q_proj[:,None,:] + v_proj) @ Vw.T + Vb   [B, T, 1]
  attn   = softmax(score, axis=1)
  ctx    = sum_t attn * values                [B, H]
  returns (ctx, attn)

Sharding: data-parallel over batch, 4 batches per core on 8 cores.
Weights replicated. No collectives.

Per-core dataflow (all matmuls in fp32r = full PE rate):
  - values streamed in t-chunks of 512 (natural layout [t,h], SWDGE
    DMA-cast fp32->fp32r); PE-transposes 128x128 blocks into [h,t]
    layout for the projection matmul; the natural tiles stay resident
    in SBUF and are reused by the context matmul (values read from HBM
    exactly once).
  - v_projT accumulated per 128-o-tile in PSUM (WvT stationary);
    ScalarE applies tanh + (q_projT + bq + bv) bias directly
    PSUM->SBUF; score matmul (M=1) accumulates over o-tiles.
  - exact softmax per batch: reduce_max over [1,2048] scores, single
    ScalarE Exp with accum_out giving the denominator.
  - context matmul: attn column tiles (PE outer-product transpose)
    against resident natural value tiles, contraction over t in PSUM.
"""

import sys

sys.path.insert(0, "/opt/trn_rl_repo")

import numpy as np

import concourse.bass as bass
import concourse.mybir as mybir
import concourse.tile as tile
from concourse import bacc, bass_utils
from concourse.masks import make_identity

dt = mybir.dt
AF = mybir.ActivationFunctionType

B, T, H = 32, 2048, 1024
N_CORES = 8
BPC = B // N_CORES          # batches per core
P = 128
HT = H // P                 # 8 h-tiles (contraction)
OT = H // P                 # 8 o-tiles (projection output)
CHUNK = 512
NCHUNK = T // CHUNK         # 4 chunks per batch
SUB = CHUNK // P            # 4 t-subtiles per chunk
TS = T // P                 # 16 t-subtiles per batch
NAT_BUFS = 20               # resident naturals (16 per batch) + prefetch
REPEAT = 1                  # benchmark-only: replicate the main loop


def build(nc):
    q_d = nc.dram_tensor("query", [BPC, H], dt.float32, kind="ExternalInput").ap()
    v_d = nc.dram_tensor("values", [BPC, T, H], dt.float32, kind="ExternalInput").ap()
    wq_d = nc.dram_tensor("Wq", [H, H], dt.float32, kind="ExternalInput").ap()
    bq_d = nc.dram_tensor("bq", [H], dt.float32, kind="ExternalInput").ap()
    wv_d = nc.dram_tensor("Wv", [H, H], dt.float32, kind="ExternalInput").ap()
    bv_d = nc.dram_tensor("bv", [H], dt.float32, kind="ExternalInput").ap()
    vw_d = nc.dram_tensor("Vw", [1, H], dt.float32, kind="ExternalInput").ap()
    vb_d = nc.dram_tensor("Vb", [1], dt.float32, kind="ExternalInput").ap()
    ctx_d = nc.dram_tensor("ctx_out", [BPC, H], dt.float32, kind="ExternalOutput").ap()
    attn_d = nc.dram_tensor("attn_out", [BPC, T], dt.float32, kind="ExternalOutput").ap()

    with tile.TileContext(nc) as tc:
        with (
            tc.tile_pool(name="nat", bufs=NAT_BUFS) as nat_p,
            tc.tile_pool(name="valT", bufs=2) as valT_p,
            tc.tile_pool(name="wT", bufs=1) as wT_p,
            tc.tile_pool(name="tanh", bufs=3) as tanh_p,
            tc.tile_pool(name="row", bufs=1) as row_p,
            tc.tile_pool(name="smalls", bufs=1) as small_p,
            tc.tile_pool(name="ps_tp", bufs=2, space="PSUM") as ps_tp,
            tc.tile_pool(name="ps_vp", bufs=2, space="PSUM") as ps_vp,
            tc.tile_pool(name="ps_sc", bufs=1, space="PSUM") as ps_sc,
            tc.tile_pool(name="ps_at", bufs=1, space="PSUM") as ps_at,
            tc.tile_pool(name="ps_ctx", bufs=1, space="PSUM") as ps_ctx,
        ):
            # ---------- constants ----------
            ident_f = small_p.tile([P, P], dt.float32, tag="ident_f")
            make_identity(nc, ident_f[:])
            ident_r = small_p.tile([P, P], dt.float32r, tag="ident_r")
            nc.vector.tensor_copy(ident_r[:], ident_f[:])
            ones_t = small_p.tile([1, 1], dt.float32, tag="ones")
            nc.vector.memset(ones_t[:], 1.0)
            vb_t = small_p.tile([1, 1], dt.float32, tag="vb")
            nc.sync.dma_start(vb_t[:], vb_d[None, :])

            # biases in transposed o-layout [128, OT] (o = ot*128 + p)
            bq_t = small_p.tile([P, OT], dt.float32, tag="bq")
            nc.sync.dma_start(bq_t[:], bq_d.rearrange("(ot p) -> p ot", p=P))
            bv_t = small_p.tile([P, OT], dt.float32, tag="bv")
            nc.sync.dma_start(bv_t[:], bv_d.rearrange("(ot p) -> p ot", p=P))
            bqbv = small_p.tile([P, OT], dt.float32, tag="bqbv")
            nc.vector.tensor_add(bqbv[:], bq_t[:], bv_t[:])

            # VwT in [128, OT] layout, rounded to fp32r
            vwT_f = small_p.tile([P, OT], dt.float32, tag="vwT_f")
            nc.sync.dma_start(vwT_f[:], vw_d.rearrange("one (ot p) -> p (one ot)", p=P))
            vwT = small_p.tile([P, OT], dt.float32r, tag="vwT")
            nc.vector.tensor_copy(vwT[:], vwT_f[:])

            # queryT [128, HT, BPC] fp32r (h = hk*128 + p)
            qT = small_p.tile([P, HT, BPC], dt.float32r, tag="qT")
            for b in range(BPC):
                nc.gpsimd.dma_start(
                    qT[:, :, b], q_d[b].rearrange("(hk p) -> p hk", p=P)
                )

            # ---------- weight transposes ----------
            def load_wT(w_dram, tagname):
                tiles = []
                for hk in range(HT):
                    wt = wT_p.tile([P, H], dt.float32r, tag=f"{tagname}{hk}",
                                   name=f"{tagname}{hk}")
                    tiles.append(wt)
                for ot in range(OT):
                    wn = nat_p.tile([P, H], dt.float32r, tag="nat", name="wnat")
                    nc.gpsimd.dma_start(wn[:], w_dram[ot * P:(ot + 1) * P, :])
                    for hk in range(HT):
                        tp = ps_tp.tile([P, P], dt.float32r, tag="tp", name="wtp")
                        nc.tensor.transpose(tp[:], wn[:, hk * P:(hk + 1) * P], ident_r[:])
                        nc.vector.tensor_copy(tiles[hk][:, ot * P:(ot + 1) * P], tp[:])
                return tiles

            # ---------- q_proj + bias addend [128, OT, BPC] ----------
            # WqT blocks are transposed on the fly into a rotating buffer
            # (no resident WqT: saves 32KB/partition of SBUF).
            addend = small_p.tile([P, OT, BPC], dt.float32, tag="addend")
            wq_nats = []
            for ot in range(OT):
                wqn = nat_p.tile([P, H], dt.float32r, tag="nat", name="wqn")
                nc.gpsimd.dma_start(wqn[:], wq_d[ot * P:(ot + 1) * P, :])
                wq_nats.append(wqn)
            for ot in range(OT):
                qp = ps_vp.tile([P, BPC], dt.float32, tag="vp", name="qp")
                for hk in range(HT):
                    tp = ps_tp.tile([P, P], dt.float32r, tag="tp", name="wtp")
                    nc.tensor.transpose(
                        tp[:], wq_nats[ot][:, hk * P:(hk + 1) * P], ident_r[:]
                    )
                    wqt = valT_p.tile([P, P], dt.float32r, tag="wqt", name="wqt")
                    nc.vector.tensor_copy(wqt[:], tp[:])
                    nc.tensor.matmul(
                        qp[:], wqt[:], qT[:, hk, :],
                        start=(hk == 0), stop=(hk == HT - 1),
                    )
                nc.vector.tensor_scalar(
                    addend[:, ot, :], qp[:], bqbv[:, ot:ot + 1], None,
                    op0=mybir.AluOpType.add,
                )

            wvT = load_wT(wv_d, "wvT")

            # ---------- main loop ----------
            deferred = [None]

            def phase_a(b):
                """Returns (scores tile [1, T], list of natural tiles)."""
                scores = row_p.tile([1, T], dt.float32, tag="scores", name="scores")
                nats = []
                for c in range(NCHUNK):
                    cn = []
                    for s in range(SUB):
                        t0 = c * CHUNK + s * P
                        vn = nat_p.tile([P, H], dt.float32r, tag="nat", name="vnat")
                        nc.gpsimd.dma_start(vn[:], v_d[b, t0:t0 + P, :])
                        cn.append(vn)
                    nats.extend(cn)

                    # transpose chunk into [h, t] tiles
                    vT = []
                    for hk in range(HT):
                        vt = valT_p.tile([P, CHUNK], dt.float32r, tag=f"vT{hk}",
                                         name=f"vT{hk}")
                        tp = ps_tp.tile([P, CHUNK], dt.float32r, tag="tp", name="vtp")
                        for s in range(SUB):
                            nc.tensor.transpose(
                                tp[:, s * P:(s + 1) * P],
                                cn[s][:, hk * P:(hk + 1) * P],
                                ident_r[:],
                            )
                        nc.vector.tensor_copy(vt[:], tp[:])
                        vT.append(vt)

                    # projection + tanh + score
                    sc = ps_sc.tile([1, CHUNK], dt.float32, tag="sc", name="sc")
                    for ot in range(OT):
                        vp = ps_vp.tile([P, CHUNK], dt.float32, tag="vp", name="vp")
                        for hk in range(HT):
                            nc.tensor.matmul(
                                vp[:], wvT[hk][:, ot * P:(ot + 1) * P], vT[hk][:],
                                start=(hk == 0), stop=(hk == HT - 1),
                            )
                        th = tanh_p.tile([P, CHUNK], dt.float32r, tag="th", name="th")
                        nc.scalar.activation(
                            th[:], vp[:], AF.Tanh, bias=addend[:, ot, b:b + 1],
                        )
                        nc.tensor.matmul(
                            sc[:], vwT[:, ot:ot + 1], th[:],
                            start=(ot == 0), stop=(ot == OT - 1),
                        )
                    nc.vector.tensor_copy(scores[:, c * CHUNK:(c + 1) * CHUNK], sc[:])

                    if c == 0 and deferred[0] is not None:
                        deferred[0]()
                        deferred[0] = None
                return scores, nats

            def phase_b_scalar(b, scores):
                m_t = row_p.tile([1, 1], dt.float32, tag="m", name="m")
                nc.vector.reduce_max(m_t[:], scores[:], axis=mybir.AxisListType.X)
                bias_t = row_p.tile([1, 1], dt.float32, tag="bias", name="bias")
                nc.vector.tensor_scalar(
                    bias_t[:], vb_t[:], m_t[:], None, op0=mybir.AluOpType.subtract,
                )
                attn_u = row_p.tile([1, T], dt.float32, tag="attn_u", name="attn_u")
                den = row_p.tile([1, 1], dt.float32, tag="den", name="den")
                nc.scalar.activation(
                    attn_u[:], scores[:], AF.Exp, bias=bias_t[:], accum_out=den[:],
                )
                rden = row_p.tile([1, 1], dt.float32, tag="rden", name="rden")
                nc.vector.reciprocal(rden[:], den[:])
                return attn_u, rden

            def phase_b_pe(b, attn_u, rden, nats):
                at_ps = ps_at.tile([P, TS], dt.float32, tag="at", name="at")
                for ts in range(TS):
                    nc.tensor.matmul(
                        at_ps[:, ts:ts + 1], attn_u[:, ts * P:(ts + 1) * P], ones_t[:],
                        start=True, stop=True,
                    )
                atT = row_p.tile([P, TS], dt.float32r, tag="atT", name="atT")
                nc.vector.tensor_copy(atT[:], at_ps[:])

                cps = []
                for hh in range(2):
                    cp = ps_ctx.tile([1, 512], dt.float32, tag=f"ctx{hh}", name=f"ctx{hh}")
                    cps.append(cp)
                for ts in range(TS):
                    for hh in range(2):
                        nc.tensor.matmul(
                            cps[hh][:], atT[:, ts:ts + 1],
                            nats[ts][:, hh * 512:(hh + 1) * 512],
                            start=(ts == 0), stop=(ts == TS - 1),
                        )
                ctx_sb = row_p.tile([1, H], dt.float32, tag="ctx_sb", name="ctx_sb")
                for hh in range(2):
                    nc.vector.tensor_scalar(
                        ctx_sb[:, hh * 512:(hh + 1) * 512], cps[hh][:], rden[:], None,
                        op0=mybir.AluOpType.mult,
                    )
                nc.sync.dma_start(ctx_d[b:b + 1, :], ctx_sb[:])
                attn_n = row_p.tile([1, T], dt.float32, tag="attn_n", name="attn_n")
                nc.vector.tensor_scalar(
                    attn_n[:], attn_u[:], rden[:], None, op0=mybir.AluOpType.mult,
                )
                nc.sync.dma_start(attn_d[b:b + 1, :], attn_n[:])

            for _rep in range(REPEAT):
                for b in range(BPC):
                    scores, nats = phase_a(b)
                    attn_u, rden = phase_b_scalar(b, scores)
                    deferred[0] = (
                        lambda b=b, attn_u=attn_u, rden=rden, nats=nats:
                        phase_b_pe(b, attn_u, rden, nats)
                    )
            deferred[0]()
            deferred[0] = None

    return nc


_cached = None


def _get_nc():
    global _cached
    if _cached is None:
        nc = bacc.Bacc("TRN2", target_bir_lowering=False, debug=False,
                       num_devices=N_CORES)
        build(nc)
        nc.compile()
        _cached = nc
    return _cached


def kernel(query, values, Wq, bq, Wv, bv, Vw, Vb):
    query = np.ascontiguousarray(query, dtype=np.float32)
    values = np.ascontiguousarray(values, dtype=np.float32)
    nc = _get_nc()
    in_maps = []
    for k in range(N_CORES):
        sl = slice(k * BPC, (k + 1) * BPC)
        in_maps.append({
            "query": query[sl],
            "values": values[sl],
            "Wq": np.ascontiguousarray(Wq, dtype=np.float32),
            "bq": np.ascontiguousarray(bq, dtype=np.float32),
            "Wv": np.ascontiguousarray(Wv, dtype=np.float32),
            "bv": np.ascontiguousarray(bv, dtype=np.float32),
            "Vw": np.ascontiguousarray(Vw, dtype=np.float32),
            "Vb": np.ascontiguousarray(Vb, dtype=np.float32),
        })
    res = bass_utils.run_bass_kernel_spmd(nc, in_maps, list(range(N_CORES)))
    ctx = np.concatenate([res.results[k]["ctx_out"] for k in range(N_CORES)], axis=0)
    attn = np.concatenate([res.results[k]["attn_out"] for k in range(N_CORES)], axis=0)
    return ctx, attn[:, :, None]
